# revision 1
# baseline (speedup 1.0000x reference)
"""Trainium2 Bass kernel for nn_DigitCap (capsule DigitCaps layer).

Math: the reference's routing loop is degenerate — softmax over a size-1
axis is exactly 1.0, so c_ij == 1 on every iteration and the output only
depends on s[b,l,o] = sum_{p,n} W[0,p,l,o,n] * x[b,n,p], followed by the
squash nonlinearity (norm taken over the L axis, faithful to the source):

    m2[b,o]    = sum_l s[b,l,o]^2
    out[b,l,o] = s[b,l,o] * sqrt(m2[b,o]) / (1 + m2[b,o])

This collapses to one (256 x 9216) @ (9216 x 160) matmul plus a tiny
elementwise epilogue.

Sharding over 8 NeuronCores — shipped mode "bp2", batch-parallel with NO
collective: on this stack every 8-rank collective costs 50-65us of ncfw
control-plane latency regardless of payload (measured AR/AG/RS/A2A), which
dwarfs the extra DMA of replicating W.  So each core takes 32 batch rows,
reads all of W (5.9 MB) plus its 1.2 MB x-slice, and no cross-core
communication happens at all.

PE efficiency at M=32 is recovered with 4x column tiling: each PE pass
runs 4 K-chunks concurrently in the four 32-column groups of the array
(tile_position=(0,32j)), accumulating into four disjoint 32-partition
strips of one PSUM tile; the strips are then combined with a tiny
selection-matrix matmul (DVE cannot add across base partitions).  Inputs
are host-packed so each pass's W/xt tile is one contiguous DRAM block,
and every W pass-load is split across both HWDGE queues (sync+scalar),
which lifts aggregate DMA from ~190 to ~245 GB/s — the matmul phase is
DMA-bandwidth-bound (7.1 MB/core through the LNC1-shared HBM port).

Alternate modes kept for reference (all measured slower): "bp" (unpacked
batch-parallel, 52us), "a2a" (K-sharded + AllToAll, 87us), "rs"/"ar"/"ag"
(K-sharded + ReduceScatter/AllReduce/AllGather, 96-105us); shipped bp2
measures ~38us end-to-end on hardware (paired DMA issues, SBUF-resident
x loaded by two early DMAs, PE warm-up
matmuls during the load lead-in, balanced queue alternation).

The free dim everywhere is ordered f = o*10 + l so the squash l-reduction
is an innermost-axis DVE reduce; the host converts the gathered (256,160)
result back to (256, 10, 16).
"""

import numpy as np

B, N, P, L, O = 256, 8, 1152, 10, 16
NCORES = 8
KC = P // 128          # 9 k-chunks of 128 per core
BB = B // NCORES       # 32 batch rows per core in the scatter modes
LO = L * O             # 160

MODE = "bp2"

GP = 4                 # col-tiled k-chunks per PE pass in "bp" mode
NPASS = N * P // 128 // GP   # 18 passes over the full K for one core

_cache = {}


def _emit_squash(nc, mybir, post, s, nrows, idx):
    """Emit squash for an SBUF tile s of shape [nrows, LO]; returns v tile."""
    f32 = mybir.dt.float32
    sq = post.tile([nrows, LO], f32, name=f"sq{idx}")
    m2 = post.tile([nrows, O], f32, name=f"m2{idx}")
    rt = post.tile([nrows, O], f32, name=f"rt{idx}")
    dn = post.tile([nrows, O], f32, name=f"dn{idx}")
    tf = post.tile([nrows, O], f32, name=f"tf{idx}")
    vv = post.tile([nrows, LO], f32, name=f"vv{idx}")
    nc.vector.tensor_mul(sq[:], s[:], s[:])
    nc.vector.reduce_sum(
        m2[:], sq[:].rearrange("b (o l) -> b o l", l=L),
        axis=mybir.AxisListType.X)
    nc.scalar.activation(rt[:], m2[:], mybir.ActivationFunctionType.Sqrt)
    nc.vector.tensor_scalar_add(dn[:], m2[:], 1.0)
    nc.vector.reciprocal(dn[:], dn[:])
    nc.vector.tensor_mul(tf[:], rt[:], dn[:])
    nc.vector.tensor_mul(
        vv[:].rearrange("b (o l) -> b o l", l=L),
        s[:].rearrange("b (o l) -> b o l", l=L),
        tf[:][:, :, None].broadcast_to([nrows, O, L]))
    return vv


def _build(mode=MODE):
    if mode in _cache:
        return _cache[mode]

    import concourse.bacc as bacc
    import concourse.mybir as mybir
    import concourse.tile as tile

    f32 = mybir.dt.float32
    nc = bacc.Bacc("TRN2", target_bir_lowering=False, debug=False,
                   num_devices=NCORES)
    if mode == "bp":
        return _build_bp(nc, mybir)
    if mode == "bp2":
        return _build_bp2(nc, mybir)
    xt_d = nc.dram_tensor("xt", [P, B], f32, kind="ExternalInput").ap()
    w_d = nc.dram_tensor("w", [P, LO], f32, kind="ExternalInput").ap()
    out_rows = BB if mode in ("rs", "a2a") else B
    out_d = nc.dram_tensor("out", [out_rows, LO], f32,
                           kind="ExternalOutput").ap()

    with tile.TileContext(nc) as tc:
        with (
            tc.tile_pool(name="io", bufs=3) as io_pool,
            tc.tile_pool(name="ps", bufs=1, space="PSUM") as ps_pool,
            tc.tile_pool(name="dram", bufs=1, space="DRAM") as dram_pool,
            tc.tile_pool(name="post", bufs=1) as post,
        ):
            xt_v = xt_d.rearrange("(c p) b -> c p b", p=128)
            w_v = w_d.rearrange("(c p) f -> c p f", p=128)
            ps0 = ps_pool.tile([128, LO], f32, name="ps0")
            ps1 = ps_pool.tile([128, LO], f32, name="ps1")
            for c in range(KC):
                xt_t = io_pool.tile([128, B], f32, tag="xt", name=f"xt{c}")
                w_t = io_pool.tile([128, LO], f32, tag="w", name=f"w{c}")
                nc.sync.dma_start(xt_t[:], xt_v[c])
                nc.sync.dma_start(w_t[:], w_v[c])
                nc.tensor.matmul(ps0[:], xt_t[:, 0:128], w_t[:],
                                 start=(c == 0), stop=(c == KC - 1))
                nc.tensor.matmul(ps1[:], xt_t[:, 128:256], w_t[:],
                                 start=(c == 0), stop=(c == KC - 1))

            partial = dram_pool.tile([B, LO], f32, name="partial")
            s0 = post.tile([128, LO], f32, name="s0")
            s1 = post.tile([128, LO], f32, name="s1")
            nc.vector.tensor_copy(s0[:], ps0[:])
            nc.vector.tensor_copy(s1[:], ps1[:])
            nc.sync.dma_start(partial[0:128, :], s0[:])
            nc.sync.dma_start(partial[128:256, :], s1[:])

            rg = [list(range(NCORES))]
            if mode == "ar":
                red = dram_pool.tile([B, LO], f32, name="red",
                                     addr_space="Shared")
                nc.gpsimd.collective_compute(
                    "AllReduce", mybir.AluOpType.add, replica_groups=rg,
                    ins=[partial.opt()], outs=[red.opt()])
                for h in range(2):
                    sh = post.tile([128, LO], f32, name=f"sh{h}")
                    nc.sync.dma_start(sh[:], red[128 * h:128 * (h + 1), :])
                    vv = _emit_squash(nc, mybir, post, sh, 128, h)
                    nc.sync.dma_start(out_d[128 * h:128 * (h + 1), :], vv[:])
            elif mode == "ag":
                red = dram_pool.tile([NCORES * B, LO], f32, name="red",
                                     addr_space="Shared")
                nc.gpsimd.collective_compute(
                    "AllGather", mybir.AluOpType.bypass, replica_groups=rg,
                    ins=[partial.opt()], outs=[red.opt()])
                red_v = red.rearrange("(r b) f -> b r f", b=B)
                for h in range(2):
                    r8 = post.tile([128, NCORES, LO], f32, name=f"r8{h}")
                    nc.sync.dma_start(r8[:], red_v[128 * h:128 * (h + 1)])
                    sh = post.tile([128, LO], f32, name=f"sh{h}")
                    nc.vector.reduce_sum(
                        sh[:], r8[:].rearrange("b r f -> b f r"),
                        axis=mybir.AxisListType.X)
                    vv = _emit_squash(nc, mybir, post, sh, 128, h)
                    nc.sync.dma_start(out_d[128 * h:128 * (h + 1), :], vv[:])
            elif mode == "rs":
                red = dram_pool.tile([BB, LO], f32, name="red")
                nc.gpsimd.collective_compute(
                    "ReduceScatter", mybir.AluOpType.add, replica_groups=rg,
                    ins=[partial.opt()], outs=[red.opt()])
                s = post.tile([BB, LO], f32, name="s")
                nc.sync.dma_start(s[:], red[:])
                vv = _emit_squash(nc, mybir, post, s, BB, 0)
                nc.sync.dma_start(out_d[:], vv[:])
            else:  # a2a
                red = dram_pool.tile([B, LO], f32, name="red")
                nc.gpsimd.collective_compute(
                    "AllToAll", mybir.AluOpType.bypass, replica_groups=rg,
                    ins=[partial.opt()], outs=[red.opt()])
                r8 = post.tile([BB, NCORES, LO], f32, name="r8")
                nc.sync.dma_start(r8[:], red.rearrange("(r b) f -> b r f",
                                                       b=BB))
                s = post.tile([BB, LO], f32, name="s")
                nc.vector.reduce_sum(
                    s[:], r8[:].rearrange("b r f -> b f r"),
                    axis=mybir.AxisListType.X)
                vv = _emit_squash(nc, mybir, post, s, BB, 0)
                nc.sync.dma_start(out_d[:], vv[:])

    nc.compile()
    _cache[mode] = nc
    return nc


def _build_bp(nc, mybir):
    """Batch-parallel: W replicated, batch sharded 8 x 32, no collective.

    PE efficiency at M=32 is recovered with 4x column tiling: each PE pass
    runs 4 k-chunks concurrently in the four 32-column groups of the array,
    accumulating into four disjoint 32-partition strips of one PSUM tile.
    The four strips are partial K-sums, added together on DVE at the end.
    DMA is split across both HWDGE queues (sync + scalar)."""
    import concourse.tile as tile

    f32 = mybir.dt.float32
    K = N * P
    xt_d = nc.dram_tensor("xt", [K, BB], f32, kind="ExternalInput").ap()
    w_d = nc.dram_tensor("w", [K, LO], f32, kind="ExternalInput").ap()
    sel_d = nc.dram_tensor("sel", [128, BB], f32, kind="ExternalInput").ap()
    out_d = nc.dram_tensor("out", [BB, LO], f32, kind="ExternalOutput").ap()

    with tile.TileContext(nc) as tc:
        with (
            tc.tile_pool(name="io", bufs=3) as io_pool,
            tc.tile_pool(name="ps", bufs=1, space="PSUM") as ps_pool,
            tc.tile_pool(name="post", bufs=1) as post,
        ):
            xt_v = xt_d.rearrange("(g j p) m -> g p j m", j=GP, p=128)
            w_v = w_d.rearrange("(g j p) f -> g p j f", j=GP, p=128)
            sel_t = post.tile([128, BB], f32, name="sel_t")
            nc.scalar.dma_start(sel_t[:], sel_d[:])
            ps = ps_pool.tile([128, LO], f32, name="ps")
            for g in range(NPASS):
                xt_t = io_pool.tile([128, GP, BB], f32, tag="xt",
                                    name=f"xt{g}")
                w_t = io_pool.tile([128, GP, LO], f32, tag="w", name=f"w{g}")
                dma_eng = nc.sync if g % 2 == 0 else nc.scalar
                xt_eng = nc.scalar if g % 2 == 0 else nc.sync
                xt_eng.dma_start(xt_t[:], xt_v[g])
                dma_eng.dma_start(w_t[:], w_v[g])
                for j in range(GP):
                    nc.tensor.matmul(
                        ps[32 * j:32 * (j + 1), :], xt_t[:, j, :],
                        w_t[:, j, :], start=(g == 0), stop=(g == NPASS - 1),
                        tile_position=(0, 32 * j))

            # sum the four 32-partition strips: s = sel.T @ sp on the PE
            # (DVE cannot add across base partitions; walrus rejects it).
            sp = post.tile([128, LO], f32, name="sp")
            nc.vector.tensor_copy(sp[:], ps[:])
            ps2 = ps_pool.tile([BB, LO], f32, name="ps2")
            nc.tensor.matmul(ps2[:], sel_t[:], sp[:], start=True, stop=True)
            s = post.tile([BB, LO], f32, name="s")
            nc.vector.tensor_copy(s[:], ps2[:])
            vv = _emit_squash(nc, mybir, post, s, BB, 0)
            nc.sync.dma_start(out_d[:], vv[:])

    nc.compile()
    _cache["bp"] = nc
    return nc


def _build_bp2(nc, mybir):
    """Like bp, but inputs are host-packed so each PE pass's W/xt tile is a
    contiguous DRAM block (per-partition runs of 1280B/512B instead of
    640B/128B), and every W pass-load is split across both HWDGE queues."""
    import concourse.tile as tile

    f32 = mybir.dt.float32
    xt_d = nc.dram_tensor("xt", [128, NPASS * GP * BB], f32,
                          kind="ExternalInput").ap()
    w_d = nc.dram_tensor("w", [NPASS * 128, GP * LO], f32,
                         kind="ExternalInput").ap()
    sel_d = nc.dram_tensor("sel", [128, BB], f32, kind="ExternalInput").ap()
    out_d = nc.dram_tensor("out", [BB, LO], f32, kind="ExternalOutput").ap()

    with tile.TileContext(nc) as tc:
        with (
            tc.tile_pool(name="io", bufs=5) as io_pool,
            tc.tile_pool(name="ps", bufs=1, space="PSUM") as ps_pool,
            tc.tile_pool(name="post", bufs=1) as post,
        ):
            # DMA granularity: PR passes per issue (fewer, larger transfers —
            # each dma_start costs ~670ns of issue time on its HWDGE engine,
            # and the kernel-teardown sem storm scales with instruction count).
            # The first group is a single pass so the PE can start sooner.
            PR = 3
            groups = [1] + [PR] * ((NPASS - 1) // PR) + \
                     ([NPASS - 1 - (NPASS - 1) // PR * PR] or [])
            groups = [n for n in groups if n]
            w_vp = w_d.rearrange("(g p) f -> g p f", p=128)
            sel_t = post.tile([128, BB], f32, name="sel_t")
            nc.scalar.dma_start(sel_t[:], sel_d[:])
            # x is tiny (9.2KB/partition): keep it SBUF-resident, loaded by
            # two early DMAs instead of one per group — fewer issues and no
            # xt dependency in the W streaming pipeline.
            XA = 7 * GP * BB
            xt_all = post.tile([128, NPASS * GP * BB], f32, name="xt_all")
            nc.scalar.dma_start(xt_all[:, 0:XA], xt_d[:, 0:XA])
            ps = ps_pool.tile([128, LO], f32, name="ps")
            # PE warm-up: ~4us of dummy matmuls on the tiny sel tile while
            # the first W loads are in flight, so the HAM un-throttles the
            # PE clock (1.2 -> 2.4 GHz) before the real passes start.
            warm = ps_pool.tile([BB, BB], f32, name="warm")
            for _ in range(10):
                nc.tensor.matmul(warm[:], sel_t[:, 0:BB], sel_t[:, 0:BB],
                                 start=True, stop=True)
            g0 = 0
            for gi, npg in enumerate(groups):
                w_t = io_pool.tile([128, npg, GP * LO], f32, tag="w",
                                   name=f"w{gi}")
                ws = w_vp[g0:g0 + npg].rearrange("h p f -> p h f")
                e0, e1 = (nc.sync, nc.scalar) if gi % 2 == 0 else \
                         (nc.scalar, nc.sync)
                if npg == 1:
                    half = GP * LO // 2
                    e0.dma_start(w_t[:, 0, 0:half], ws[:, 0, 0:half])
                    e1.dma_start(w_t[:, 0, half:], ws[:, 0, half:])
                else:
                    # first-needed pass on e0, rest on e1
                    e0.dma_start(w_t[:, 0:1, :], ws[:, 0:1, :])
                    e1.dma_start(w_t[:, 1:npg, :], ws[:, 1:npg, :])
                if gi == 0:
                    nc.sync.dma_start(xt_all[:, XA:], xt_d[:, XA:])
                for h in range(npg):
                    g = g0 + h
                    for j in range(GP):
                        c = g * GP + j
                        nc.tensor.matmul(
                            ps[32 * j:32 * (j + 1), :],
                            xt_all[:, BB * c:BB * (c + 1)],
                            w_t[:, h, LO * j:LO * (j + 1)],
                            start=(g == 0), stop=(g == NPASS - 1),
                            tile_position=(0, 32 * j))
                g0 += npg

            sp = post.tile([128, LO], f32, name="sp")
            nc.vector.tensor_copy(sp[:], ps[:])
            ps2 = ps_pool.tile([BB, LO], f32, name="ps2")
            nc.tensor.matmul(ps2[:], sel_t[:], sp[:], start=True, stop=True)
            s = post.tile([BB, LO], f32, name="s")
            nc.vector.tensor_copy(s[:], ps2[:])
            vv = _emit_squash(nc, mybir, post, s, BB, 0)
            nc.sync.dma_start(out_d[:], vv[:])

    nc.compile()
    _cache["bp2"] = nc
    return nc


def _prep_inputs(x, W, mode=MODE):
    x = np.asarray(x, dtype=np.float32)
    W = np.asarray(W, dtype=np.float32)
    if mode == "bp2":
        # pack so each pass's tile is one contiguous DRAM block:
        # packed[g, p, j*D+d] = flat[128*(GP*g+j)+p, d]
        wf = np.ascontiguousarray(
            W[0].transpose(3, 0, 2, 1).reshape(N * P, LO))
        w2 = np.ascontiguousarray(
            wf.reshape(NPASS, GP, 128, LO).transpose(0, 2, 1, 3)
            .reshape(NPASS * 128, GP * LO))
        sel = np.zeros((128, BB), np.float32)
        sel[np.arange(128), np.arange(128) % BB] = 1.0
        in_maps = []
        for i in range(NCORES):
            xt = x[BB * i:BB * (i + 1)].reshape(BB, N * P).T  # (9216, 32)
            x2 = np.ascontiguousarray(
                xt.reshape(NPASS * GP, 128, BB).transpose(1, 0, 2)
                .reshape(128, NPASS * GP * BB))
            in_maps.append({"xt": x2, "w": w2, "sel": sel})
        return in_maps
    if mode == "bp":
        # xt = per-core batch-slice of x, flattened (b, n*p) and transposed;
        # w = full W with rows k=(n,p), cols f=o*10+l — identical per core.
        wf = np.ascontiguousarray(
            W[0].transpose(3, 0, 2, 1).reshape(N * P, LO))    # (9216, 160)
        sel = np.zeros((128, BB), np.float32)
        sel[np.arange(128), np.arange(128) % BB] = 1.0
        in_maps = []
        for i in range(NCORES):
            xs = x[BB * i:BB * (i + 1)].reshape(BB, N * P)
            in_maps.append({"xt": np.ascontiguousarray(xs.T), "w": wf,
                            "sel": sel})
        return in_maps
    in_maps = []
    for i in range(NCORES):
        xt = np.ascontiguousarray(x[:, i, :].T)               # (1152, 256)
        w = np.ascontiguousarray(
            W[0, :, :, :, i].transpose(0, 2, 1).reshape(P, LO))  # (1152, 160)
        in_maps.append({"xt": xt, "w": w})
    return in_maps


def _postprocess(results, mode=MODE):
    if mode in ("rs", "a2a", "bp", "bp2"):
        full = np.concatenate([results[i]["out"] for i in range(NCORES)],
                              axis=0)
    else:
        full = results[0]["out"]
    return np.ascontiguousarray(
        full.reshape(B, O, L).transpose(0, 2, 1))             # (256, 10, 16)


def kernel(x, W):
    from concourse.bass_utils import run_bass_kernel_spmd

    nc = _build(MODE)
    res = run_bass_kernel_spmd(nc, _prep_inputs(x, W, MODE),
                               core_ids=list(range(NCORES)))
    return _postprocess(res.results)



# revision 7
# speedup vs baseline: 1.3543x; 1.3543x over previous
"""Trainium2 Bass kernel for nn_DigitCap (capsule DigitCaps layer).

Math: the reference's routing loop is degenerate — softmax over a size-1
axis is exactly 1.0, so c_ij == 1 on every iteration and the output only
depends on s[b,l,o] = sum_{p,n} W[0,p,l,o,n] * x[b,n,p], followed by the
squash nonlinearity (norm taken over the L axis, faithful to the source):

    m2[b,o]    = sum_l s[b,l,o]^2
    out[b,l,o] = s[b,l,o] * sqrt(m2[b,o]) / (1 + m2[b,o])

This collapses to one (256 x 9216) @ (9216 x 160) matmul plus a tiny
elementwise epilogue.

Sharding over 8 NeuronCores — shipped mode "bp2", batch-parallel with NO
collective: on this stack every 8-rank collective costs 50-65us of ncfw
control-plane latency regardless of payload (measured AR/AG/RS/A2A), which
dwarfs the extra DMA of replicating W.  So each core takes 32 batch rows,
reads all of W (5.9 MB) plus its 1.2 MB x-slice, and no cross-core
communication happens at all.

PE efficiency at M=32 is recovered with 4x column tiling: each PE pass
runs 4 K-chunks concurrently in the four 32-column groups of the array
(tile_position=(0,32j)), accumulating into four disjoint 32-partition
strips of one PSUM tile; the strips are then combined with a tiny
selection-matrix matmul (DVE cannot add across base partitions).  Inputs
are host-packed so each pass's W/xt tile is one contiguous DRAM block,
and every W pass-load is split across both HWDGE queues (sync+scalar),
which lifts aggregate DMA from ~190 to ~245 GB/s — the matmul phase is
DMA-bandwidth-bound (7.1 MB/core through the LNC1-shared HBM port).

Alternate modes kept for reference (all measured slower): "bp" (unpacked
batch-parallel, 52us), "a2a" (K-sharded + AllToAll, 87us), "rs"/"ar"/"ag"
(K-sharded + ReduceScatter/AllReduce/AllGather, 96-105us); shipped bp2
measures ~38us end-to-end on hardware (paired DMA issues, SBUF-resident
x loaded by two early DMAs, PE warm-up
matmuls during the load lead-in, balanced queue alternation).

The free dim everywhere is ordered f = o*10 + l so the squash l-reduction
is an innermost-axis DVE reduce; the host converts the gathered (256,160)
result back to (256, 10, 16).
"""

import numpy as np

B, N, P, L, O = 256, 8, 1152, 10, 16
NCORES = 8
KC = P // 128          # 9 k-chunks of 128 per core
BB = B // NCORES       # 32 batch rows per core in the scatter modes
LO = L * O             # 160

MODE = "bp3"

GP = 4                 # col-tiled k-chunks per PE pass in "bp" mode
NPASS = N * P // 128 // GP   # 18 passes over the full K for one core

# bp3: 4-way batch x 2-way output-capsule sharding, bf16 inputs.
B4 = B // 4            # 64 batch rows per core
O2 = O // 2            # 8 output capsules per core
FO = O2 * L            # 80 output columns per core (f = o_local*10 + l)
KC3 = N * P // 128     # 72 k-chunks of 128
GP3 = 2                # col-tiled k-chunks per PE pass (two 64-col groups)
NP3 = KC3 // GP3       # 36 passes
WXC = GP3 * FO + GP3 * B4   # 288 packed cols per pass: [w | xt]

_cache = {}


def _emit_squash(nc, mybir, post, s, nrows, idx, no=O):
    """Emit squash for an SBUF tile s of shape [nrows, no*L]; returns v tile."""
    f32 = mybir.dt.float32
    nf = no * L
    sq = post.tile([nrows, nf], f32, name=f"sq{idx}")
    m2 = post.tile([nrows, no], f32, name=f"m2{idx}")
    rt = post.tile([nrows, no], f32, name=f"rt{idx}")
    dn = post.tile([nrows, no], f32, name=f"dn{idx}")
    tf = post.tile([nrows, no], f32, name=f"tf{idx}")
    vv = post.tile([nrows, nf], f32, name=f"vv{idx}")
    nc.vector.tensor_mul(sq[:], s[:], s[:])
    nc.vector.reduce_sum(
        m2[:], sq[:].rearrange("b (o l) -> b o l", l=L),
        axis=mybir.AxisListType.X)
    nc.scalar.activation(rt[:], m2[:], mybir.ActivationFunctionType.Sqrt)
    nc.vector.tensor_scalar_add(dn[:], m2[:], 1.0)
    nc.vector.reciprocal(dn[:], dn[:])
    nc.vector.tensor_mul(tf[:], rt[:], dn[:])
    nc.vector.tensor_mul(
        vv[:].rearrange("b (o l) -> b o l", l=L),
        s[:].rearrange("b (o l) -> b o l", l=L),
        tf[:][:, :, None].broadcast_to([nrows, no, L]))
    return vv


def _build(mode=MODE):
    if mode in _cache:
        return _cache[mode]

    import concourse.bacc as bacc
    import concourse.mybir as mybir
    import concourse.tile as tile

    f32 = mybir.dt.float32
    nc = bacc.Bacc("TRN2", target_bir_lowering=False, debug=False,
                   num_devices=NCORES)
    if mode == "bp":
        return _build_bp(nc, mybir)
    if mode == "bp2":
        return _build_bp2(nc, mybir)
    if mode == "bp3":
        return _build_bp3(nc, mybir)
    xt_d = nc.dram_tensor("xt", [P, B], f32, kind="ExternalInput").ap()
    w_d = nc.dram_tensor("w", [P, LO], f32, kind="ExternalInput").ap()
    out_rows = BB if mode in ("rs", "a2a") else B
    out_d = nc.dram_tensor("out", [out_rows, LO], f32,
                           kind="ExternalOutput").ap()

    with tile.TileContext(nc) as tc:
        with (
            tc.tile_pool(name="io", bufs=3) as io_pool,
            tc.tile_pool(name="ps", bufs=1, space="PSUM") as ps_pool,
            tc.tile_pool(name="dram", bufs=1, space="DRAM") as dram_pool,
            tc.tile_pool(name="post", bufs=1) as post,
        ):
            xt_v = xt_d.rearrange("(c p) b -> c p b", p=128)
            w_v = w_d.rearrange("(c p) f -> c p f", p=128)
            ps0 = ps_pool.tile([128, LO], f32, name="ps0")
            ps1 = ps_pool.tile([128, LO], f32, name="ps1")
            for c in range(KC):
                xt_t = io_pool.tile([128, B], f32, tag="xt", name=f"xt{c}")
                w_t = io_pool.tile([128, LO], f32, tag="w", name=f"w{c}")
                nc.sync.dma_start(xt_t[:], xt_v[c])
                nc.sync.dma_start(w_t[:], w_v[c])
                nc.tensor.matmul(ps0[:], xt_t[:, 0:128], w_t[:],
                                 start=(c == 0), stop=(c == KC - 1))
                nc.tensor.matmul(ps1[:], xt_t[:, 128:256], w_t[:],
                                 start=(c == 0), stop=(c == KC - 1))

            partial = dram_pool.tile([B, LO], f32, name="partial")
            s0 = post.tile([128, LO], f32, name="s0")
            s1 = post.tile([128, LO], f32, name="s1")
            nc.vector.tensor_copy(s0[:], ps0[:])
            nc.vector.tensor_copy(s1[:], ps1[:])
            nc.sync.dma_start(partial[0:128, :], s0[:])
            nc.sync.dma_start(partial[128:256, :], s1[:])

            rg = [list(range(NCORES))]
            if mode == "ar":
                red = dram_pool.tile([B, LO], f32, name="red",
                                     addr_space="Shared")
                nc.gpsimd.collective_compute(
                    "AllReduce", mybir.AluOpType.add, replica_groups=rg,
                    ins=[partial.opt()], outs=[red.opt()])
                for h in range(2):
                    sh = post.tile([128, LO], f32, name=f"sh{h}")
                    nc.sync.dma_start(sh[:], red[128 * h:128 * (h + 1), :])
                    vv = _emit_squash(nc, mybir, post, sh, 128, h)
                    nc.sync.dma_start(out_d[128 * h:128 * (h + 1), :], vv[:])
            elif mode == "ag":
                red = dram_pool.tile([NCORES * B, LO], f32, name="red",
                                     addr_space="Shared")
                nc.gpsimd.collective_compute(
                    "AllGather", mybir.AluOpType.bypass, replica_groups=rg,
                    ins=[partial.opt()], outs=[red.opt()])
                red_v = red.rearrange("(r b) f -> b r f", b=B)
                for h in range(2):
                    r8 = post.tile([128, NCORES, LO], f32, name=f"r8{h}")
                    nc.sync.dma_start(r8[:], red_v[128 * h:128 * (h + 1)])
                    sh = post.tile([128, LO], f32, name=f"sh{h}")
                    nc.vector.reduce_sum(
                        sh[:], r8[:].rearrange("b r f -> b f r"),
                        axis=mybir.AxisListType.X)
                    vv = _emit_squash(nc, mybir, post, sh, 128, h)
                    nc.sync.dma_start(out_d[128 * h:128 * (h + 1), :], vv[:])
            elif mode == "rs":
                red = dram_pool.tile([BB, LO], f32, name="red")
                nc.gpsimd.collective_compute(
                    "ReduceScatter", mybir.AluOpType.add, replica_groups=rg,
                    ins=[partial.opt()], outs=[red.opt()])
                s = post.tile([BB, LO], f32, name="s")
                nc.sync.dma_start(s[:], red[:])
                vv = _emit_squash(nc, mybir, post, s, BB, 0)
                nc.sync.dma_start(out_d[:], vv[:])
            else:  # a2a
                red = dram_pool.tile([B, LO], f32, name="red")
                nc.gpsimd.collective_compute(
                    "AllToAll", mybir.AluOpType.bypass, replica_groups=rg,
                    ins=[partial.opt()], outs=[red.opt()])
                r8 = post.tile([BB, NCORES, LO], f32, name="r8")
                nc.sync.dma_start(r8[:], red.rearrange("(r b) f -> b r f",
                                                       b=BB))
                s = post.tile([BB, LO], f32, name="s")
                nc.vector.reduce_sum(
                    s[:], r8[:].rearrange("b r f -> b f r"),
                    axis=mybir.AxisListType.X)
                vv = _emit_squash(nc, mybir, post, s, BB, 0)
                nc.sync.dma_start(out_d[:], vv[:])

    nc.compile()
    _cache[mode] = nc
    return nc


def _build_bp(nc, mybir):
    """Batch-parallel: W replicated, batch sharded 8 x 32, no collective.

    PE efficiency at M=32 is recovered with 4x column tiling: each PE pass
    runs 4 k-chunks concurrently in the four 32-column groups of the array,
    accumulating into four disjoint 32-partition strips of one PSUM tile.
    The four strips are partial K-sums, added together on DVE at the end.
    DMA is split across both HWDGE queues (sync + scalar)."""
    import concourse.tile as tile

    f32 = mybir.dt.float32
    K = N * P
    xt_d = nc.dram_tensor("xt", [K, BB], f32, kind="ExternalInput").ap()
    w_d = nc.dram_tensor("w", [K, LO], f32, kind="ExternalInput").ap()
    sel_d = nc.dram_tensor("sel", [128, BB], f32, kind="ExternalInput").ap()
    out_d = nc.dram_tensor("out", [BB, LO], f32, kind="ExternalOutput").ap()

    with tile.TileContext(nc) as tc:
        with (
            tc.tile_pool(name="io", bufs=3) as io_pool,
            tc.tile_pool(name="ps", bufs=1, space="PSUM") as ps_pool,
            tc.tile_pool(name="post", bufs=1) as post,
        ):
            xt_v = xt_d.rearrange("(g j p) m -> g p j m", j=GP, p=128)
            w_v = w_d.rearrange("(g j p) f -> g p j f", j=GP, p=128)
            sel_t = post.tile([128, BB], f32, name="sel_t")
            nc.scalar.dma_start(sel_t[:], sel_d[:])
            ps = ps_pool.tile([128, LO], f32, name="ps")
            for g in range(NPASS):
                xt_t = io_pool.tile([128, GP, BB], f32, tag="xt",
                                    name=f"xt{g}")
                w_t = io_pool.tile([128, GP, LO], f32, tag="w", name=f"w{g}")
                dma_eng = nc.sync if g % 2 == 0 else nc.scalar
                xt_eng = nc.scalar if g % 2 == 0 else nc.sync
                xt_eng.dma_start(xt_t[:], xt_v[g])
                dma_eng.dma_start(w_t[:], w_v[g])
                for j in range(GP):
                    nc.tensor.matmul(
                        ps[32 * j:32 * (j + 1), :], xt_t[:, j, :],
                        w_t[:, j, :], start=(g == 0), stop=(g == NPASS - 1),
                        tile_position=(0, 32 * j))

            # sum the four 32-partition strips: s = sel.T @ sp on the PE
            # (DVE cannot add across base partitions; walrus rejects it).
            sp = post.tile([128, LO], f32, name="sp")
            nc.vector.tensor_copy(sp[:], ps[:])
            ps2 = ps_pool.tile([BB, LO], f32, name="ps2")
            nc.tensor.matmul(ps2[:], sel_t[:], sp[:], start=True, stop=True)
            s = post.tile([BB, LO], f32, name="s")
            nc.vector.tensor_copy(s[:], ps2[:])
            vv = _emit_squash(nc, mybir, post, s, BB, 0)
            nc.sync.dma_start(out_d[:], vv[:])

    nc.compile()
    _cache["bp"] = nc
    return nc


def _build_bp2(nc, mybir):
    """Like bp, but inputs are host-packed so each PE pass's W/xt tile is a
    contiguous DRAM block (per-partition runs of 1280B/512B instead of
    640B/128B), and every W pass-load is split across both HWDGE queues."""
    import concourse.tile as tile

    f32 = mybir.dt.float32
    xt_d = nc.dram_tensor("xt", [128, NPASS * GP * BB], f32,
                          kind="ExternalInput").ap()
    w_d = nc.dram_tensor("w", [NPASS * 128, GP * LO], f32,
                         kind="ExternalInput").ap()
    sel_d = nc.dram_tensor("sel", [128, BB], f32, kind="ExternalInput").ap()
    out_d = nc.dram_tensor("out", [BB, LO], f32, kind="ExternalOutput").ap()

    with tile.TileContext(nc) as tc:
        with (
            tc.tile_pool(name="io", bufs=5) as io_pool,
            tc.tile_pool(name="ps", bufs=1, space="PSUM") as ps_pool,
            tc.tile_pool(name="post", bufs=1) as post,
        ):
            # DMA granularity: PR passes per issue (fewer, larger transfers —
            # each dma_start costs ~670ns of issue time on its HWDGE engine,
            # and the kernel-teardown sem storm scales with instruction count).
            # The first group is a single pass so the PE can start sooner.
            PR = 3
            groups = [1] + [PR] * ((NPASS - 1) // PR) + \
                     ([NPASS - 1 - (NPASS - 1) // PR * PR] or [])
            groups = [n for n in groups if n]
            w_vp = w_d.rearrange("(g p) f -> g p f", p=128)
            sel_t = post.tile([128, BB], f32, name="sel_t")
            nc.scalar.dma_start(sel_t[:], sel_d[:])
            # x is tiny (9.2KB/partition): keep it SBUF-resident, loaded by
            # two early DMAs instead of one per group — fewer issues and no
            # xt dependency in the W streaming pipeline.
            XA = 7 * GP * BB
            xt_all = post.tile([128, NPASS * GP * BB], f32, name="xt_all")
            nc.scalar.dma_start(xt_all[:, 0:XA], xt_d[:, 0:XA])
            ps = ps_pool.tile([128, LO], f32, name="ps")
            # PE warm-up: ~4us of dummy matmuls on the tiny sel tile while
            # the first W loads are in flight, so the HAM un-throttles the
            # PE clock (1.2 -> 2.4 GHz) before the real passes start.
            warm = ps_pool.tile([BB, BB], f32, name="warm")
            for _ in range(10):
                nc.tensor.matmul(warm[:], sel_t[:, 0:BB], sel_t[:, 0:BB],
                                 start=True, stop=True)
            g0 = 0
            for gi, npg in enumerate(groups):
                w_t = io_pool.tile([128, npg, GP * LO], f32, tag="w",
                                   name=f"w{gi}")
                ws = w_vp[g0:g0 + npg].rearrange("h p f -> p h f")
                e0, e1 = (nc.sync, nc.scalar) if gi % 2 == 0 else \
                         (nc.scalar, nc.sync)
                if npg == 1:
                    half = GP * LO // 2
                    e0.dma_start(w_t[:, 0, 0:half], ws[:, 0, 0:half])
                    e1.dma_start(w_t[:, 0, half:], ws[:, 0, half:])
                else:
                    # first-needed pass on e0, rest on e1
                    e0.dma_start(w_t[:, 0:1, :], ws[:, 0:1, :])
                    e1.dma_start(w_t[:, 1:npg, :], ws[:, 1:npg, :])
                if gi == 0:
                    nc.sync.dma_start(xt_all[:, XA:], xt_d[:, XA:])
                for h in range(npg):
                    g = g0 + h
                    for j in range(GP):
                        c = g * GP + j
                        nc.tensor.matmul(
                            ps[32 * j:32 * (j + 1), :],
                            xt_all[:, BB * c:BB * (c + 1)],
                            w_t[:, h, LO * j:LO * (j + 1)],
                            start=(g == 0), stop=(g == NPASS - 1),
                            tile_position=(0, 32 * j))
                g0 += npg

            sp = post.tile([128, LO], f32, name="sp")
            nc.vector.tensor_copy(sp[:], ps[:])
            ps2 = ps_pool.tile([BB, LO], f32, name="ps2")
            nc.tensor.matmul(ps2[:], sel_t[:], sp[:], start=True, stop=True)
            s = post.tile([BB, LO], f32, name="s")
            nc.vector.tensor_copy(s[:], ps2[:])
            vv = _emit_squash(nc, mybir, post, s, BB, 0)
            nc.sync.dma_start(out_d[:], vv[:])

    nc.compile()
    _cache["bp2"] = nc
    return nc


def _build_bp3(nc, mybir):
    """4-way batch x 2-way output-capsule sharding, bf16 inputs.

    Each core computes s[b, f] for 64 batch rows and 80 output columns
    (8 of the 16 o-capsules, all 10 l's; the squash l-reduction stays
    core-local).  Per-core traffic drops from 7.1 MB (bp2) to 2.65 MB:
    bf16 halves the bytes and the 4x2 grid replicates x only 2x and W
    only 4x instead of 8x.

    W and x are host-interleaved into ONE packed stream wx: per PE pass
    g the block [w(2 chunks, 160 cols) | xt(2 chunks, 128 cols)], so DMA
    delivery order == PE consumption order, every transfer is one
    contiguous per-partition run, and the two HWDGE queues split each
    group at a pass boundary.  M=64 PE efficiency is recovered with 2x
    column tiling (tile_position=(0,64j)); the two 64-partition strips
    are summed by a small selection-matrix matmul as in bp2.
    """
    import concourse.tile as tile

    f32 = mybir.dt.float32
    bf16 = mybir.dt.bfloat16
    wx_d = nc.dram_tensor("wx", [128, NP3 * WXC], bf16,
                          kind="ExternalInput").ap()
    sel_d = nc.dram_tensor("sel", [128, B4], f32, kind="ExternalInput").ap()
    out_d = nc.dram_tensor("out", [B4, FO], f32, kind="ExternalOutput").ap()

    with tile.TileContext(nc) as tc:
        with (
            tc.tile_pool(name="io", bufs=5) as io_pool,
            tc.tile_pool(name="ps", bufs=1, space="PSUM") as ps_pool,
            tc.tile_pool(name="post", bufs=1) as post,
        ):
            PR = 6
            groups = [2] + [PR] * ((NP3 - 2) // PR)
            rem = NP3 - sum(groups)
            if rem:
                groups.append(rem)
            wx_v = wx_d.rearrange("p (g c) -> p g c", c=WXC)
            sel_t = post.tile([128, B4], f32, name="sel_t")
            nc.scalar.dma_start(sel_t[:], sel_d[:])
            ps = ps_pool.tile([128, FO], f32, name="ps")
            # PE warm-up on the sel tile while the first loads are in
            # flight, so the HAM un-throttles the PE clock before the
            # real passes start.
            warm = ps_pool.tile([B4, B4], f32, name="warm")
            for _ in range(10):
                nc.tensor.matmul(warm[:], sel_t[:, 0:B4], sel_t[:, 0:B4],
                                 start=True, stop=True)
            g0 = 0
            for gi, npg in enumerate(groups):
                wx_t = io_pool.tile([128, npg, WXC], bf16, tag="wx",
                                    name=f"wx{gi}")
                src = wx_v[:, g0:g0 + npg]
                e0, e1 = (nc.sync, nc.scalar) if gi % 2 == 0 else \
                         (nc.scalar, nc.sync)
                ha = (npg + 1) // 2
                e0.dma_start(wx_t[:, 0:ha, :], src[:, 0:ha])
                e1.dma_start(wx_t[:, ha:npg, :], src[:, ha:npg])
                for h in range(npg):
                    g = g0 + h
                    for j in range(GP3):
                        nc.tensor.matmul(
                            ps[B4 * j:B4 * (j + 1), :],
                            wx_t[:, h, GP3 * FO + B4 * j:
                                 GP3 * FO + B4 * (j + 1)],
                            wx_t[:, h, FO * j:FO * (j + 1)],
                            start=(g == 0), stop=(g == NP3 - 1),
                            tile_position=(0, B4 * j))
                g0 += npg

            # sum the two 64-partition strips: s = sel.T @ sp on the PE
            sp = post.tile([128, FO], f32, name="sp")
            nc.vector.tensor_copy(sp[:], ps[:])
            ps2 = ps_pool.tile([B4, FO], f32, name="ps2")
            nc.tensor.matmul(ps2[:], sel_t[:], sp[:], start=True, stop=True)
            s = post.tile([B4, FO], f32, name="s")
            nc.vector.tensor_copy(s[:], ps2[:])
            vv = _emit_squash(nc, mybir, post, s, B4, 0, no=O2)
            nc.sync.dma_start(out_d[:], vv[:])

    nc.compile()
    _cache["bp3"] = nc
    return nc


def _prep_inputs(x, W, mode=MODE):
    x = np.asarray(x, dtype=np.float32)
    W = np.asarray(W, dtype=np.float32)
    if mode == "bp3":
        import ml_dtypes
        bf16 = ml_dtypes.bfloat16
        # wf rows k=(n,p), cols f=o*10+l
        wf = np.ascontiguousarray(
            W[0].transpose(3, 0, 2, 1).reshape(N * P, LO))
        sel = np.zeros((128, B4), np.float32)
        sel[np.arange(128), np.arange(128) % B4] = 1.0
        # per-pass packed blocks, shared pieces computed once
        wpass = {}
        for ci in range(2):
            wc = wf[:, FO * ci:FO * (ci + 1)].reshape(NP3, GP3 * 128, FO)
            wpass[ci] = wc.reshape(NP3, GP3, 128, FO).transpose(
                0, 2, 1, 3).reshape(NP3, 128, GP3 * FO)
        xpass = {}
        for ri in range(4):
            xt = x[B4 * ri:B4 * (ri + 1)].reshape(B4, N * P).T  # (9216, 64)
            xpass[ri] = xt.reshape(NP3, GP3, 128, B4).transpose(
                0, 2, 1, 3).reshape(NP3, 128, GP3 * B4)
        in_maps = []
        for i in range(NCORES):
            ri, ci = i // 2, i % 2
            wx = np.concatenate([wpass[ci], xpass[ri]], axis=2)
            wx = np.ascontiguousarray(
                wx.transpose(1, 0, 2).reshape(128, NP3 * WXC)).astype(bf16)
            in_maps.append({"wx": wx, "sel": sel})
        return in_maps
    if mode == "bp2":
        # pack so each pass's tile is one contiguous DRAM block:
        # packed[g, p, j*D+d] = flat[128*(GP*g+j)+p, d]
        wf = np.ascontiguousarray(
            W[0].transpose(3, 0, 2, 1).reshape(N * P, LO))
        w2 = np.ascontiguousarray(
            wf.reshape(NPASS, GP, 128, LO).transpose(0, 2, 1, 3)
            .reshape(NPASS * 128, GP * LO))
        sel = np.zeros((128, BB), np.float32)
        sel[np.arange(128), np.arange(128) % BB] = 1.0
        in_maps = []
        for i in range(NCORES):
            xt = x[BB * i:BB * (i + 1)].reshape(BB, N * P).T  # (9216, 32)
            x2 = np.ascontiguousarray(
                xt.reshape(NPASS * GP, 128, BB).transpose(1, 0, 2)
                .reshape(128, NPASS * GP * BB))
            in_maps.append({"xt": x2, "w": w2, "sel": sel})
        return in_maps
    if mode == "bp":
        # xt = per-core batch-slice of x, flattened (b, n*p) and transposed;
        # w = full W with rows k=(n,p), cols f=o*10+l — identical per core.
        wf = np.ascontiguousarray(
            W[0].transpose(3, 0, 2, 1).reshape(N * P, LO))    # (9216, 160)
        sel = np.zeros((128, BB), np.float32)
        sel[np.arange(128), np.arange(128) % BB] = 1.0
        in_maps = []
        for i in range(NCORES):
            xs = x[BB * i:BB * (i + 1)].reshape(BB, N * P)
            in_maps.append({"xt": np.ascontiguousarray(xs.T), "w": wf,
                            "sel": sel})
        return in_maps
    in_maps = []
    for i in range(NCORES):
        xt = np.ascontiguousarray(x[:, i, :].T)               # (1152, 256)
        w = np.ascontiguousarray(
            W[0, :, :, :, i].transpose(0, 2, 1).reshape(P, LO))  # (1152, 160)
        in_maps.append({"xt": xt, "w": w})
    return in_maps


def _postprocess(results, mode=MODE):
    if mode == "bp3":
        full = np.zeros((B, LO), np.float32)
        for i in range(NCORES):
            ri, ci = i // 2, i % 2
            full[B4 * ri:B4 * (ri + 1), FO * ci:FO * (ci + 1)] = \
                results[i]["out"]
        return np.ascontiguousarray(
            full.reshape(B, O, L).transpose(0, 2, 1))
    if mode in ("rs", "a2a", "bp", "bp2"):
        full = np.concatenate([results[i]["out"] for i in range(NCORES)],
                              axis=0)
    else:
        full = results[0]["out"]
    return np.ascontiguousarray(
        full.reshape(B, O, L).transpose(0, 2, 1))             # (256, 10, 16)


def kernel(x, W):
    from concourse.bass_utils import run_bass_kernel_spmd

    nc = _build(MODE)
    res = run_bass_kernel_spmd(nc, _prep_inputs(x, W, MODE),
                               core_ids=list(range(NCORES)))
    return _postprocess(res.results)



# revision 9
# speedup vs baseline: 1.4842x; 1.0959x over previous
"""Trainium2 Bass kernel for nn_DigitCap (capsule DigitCaps layer).

Math: the reference's routing loop is degenerate — softmax over a size-1
axis is exactly 1.0, so c_ij == 1 on every iteration and the output only
depends on s[b,l,o] = sum_{p,n} W[0,p,l,o,n] * x[b,n,p], followed by the
squash nonlinearity (norm taken over the L axis, faithful to the source):

    m2[b,o]    = sum_l s[b,l,o]^2
    out[b,l,o] = s[b,l,o] * sqrt(m2[b,o]) / (1 + m2[b,o])

This collapses to one (256 x 9216) @ (9216 x 160) matmul plus a tiny
elementwise epilogue.

Sharding over 8 NeuronCores — shipped mode "bp2", batch-parallel with NO
collective: on this stack every 8-rank collective costs 50-65us of ncfw
control-plane latency regardless of payload (measured AR/AG/RS/A2A), which
dwarfs the extra DMA of replicating W.  So each core takes 32 batch rows,
reads all of W (5.9 MB) plus its 1.2 MB x-slice, and no cross-core
communication happens at all.

PE efficiency at M=32 is recovered with 4x column tiling: each PE pass
runs 4 K-chunks concurrently in the four 32-column groups of the array
(tile_position=(0,32j)), accumulating into four disjoint 32-partition
strips of one PSUM tile; the strips are then combined with a tiny
selection-matrix matmul (DVE cannot add across base partitions).  Inputs
are host-packed so each pass's W/xt tile is one contiguous DRAM block,
and every W pass-load is split across both HWDGE queues (sync+scalar),
which lifts aggregate DMA from ~190 to ~245 GB/s — the matmul phase is
DMA-bandwidth-bound (7.1 MB/core through the LNC1-shared HBM port).

Alternate modes kept for reference (all measured slower): "bp" (unpacked
batch-parallel, 52us), "a2a" (K-sharded + AllToAll, 87us), "rs"/"ar"/"ag"
(K-sharded + ReduceScatter/AllReduce/AllGather, 96-105us); shipped bp2
measures ~38us end-to-end on hardware (paired DMA issues, SBUF-resident
x loaded by two early DMAs, PE warm-up
matmuls during the load lead-in, balanced queue alternation).

The free dim everywhere is ordered f = o*10 + l so the squash l-reduction
is an innermost-axis DVE reduce; the host converts the gathered (256,160)
result back to (256, 10, 16).
"""

import numpy as np

B, N, P, L, O = 256, 8, 1152, 10, 16
NCORES = 8
KC = P // 128          # 9 k-chunks of 128 per core
BB = B // NCORES       # 32 batch rows per core in the scatter modes
LO = L * O             # 160

MODE = "bp3"

GP = 4                 # col-tiled k-chunks per PE pass in "bp" mode
NPASS = N * P // 128 // GP   # 18 passes over the full K for one core

# bp3: 4-way batch x 2-way output-capsule sharding, bf16 inputs.
B4 = B // 4            # 64 batch rows per core
O2 = O // 2            # 8 output capsules per core
FO = O2 * L            # 80 output columns per core (f = o_local*10 + l)
KC3 = N * P // 128     # 72 k-chunks of 128
GP3 = 2                # col-tiled k-chunks per PE pass (two 64-col groups)
NP3 = KC3 // GP3       # 36 passes
WXC = GP3 * FO + GP3 * B4   # 288 packed cols per pass: [w | xt]

_cache = {}


def _emit_squash(nc, mybir, post, s, nrows, idx, no=O):
    """Emit squash for an SBUF tile s of shape [nrows, no*L]; returns v tile."""
    f32 = mybir.dt.float32
    nf = no * L
    sq = post.tile([nrows, nf], f32, name=f"sq{idx}")
    m2 = post.tile([nrows, no], f32, name=f"m2{idx}")
    rt = post.tile([nrows, no], f32, name=f"rt{idx}")
    dn = post.tile([nrows, no], f32, name=f"dn{idx}")
    tf = post.tile([nrows, no], f32, name=f"tf{idx}")
    vv = post.tile([nrows, nf], f32, name=f"vv{idx}")
    nc.vector.tensor_mul(sq[:], s[:], s[:])
    nc.vector.reduce_sum(
        m2[:], sq[:].rearrange("b (o l) -> b o l", l=L),
        axis=mybir.AxisListType.X)
    nc.scalar.activation(rt[:], m2[:], mybir.ActivationFunctionType.Sqrt)
    nc.vector.tensor_scalar_add(dn[:], m2[:], 1.0)
    nc.vector.reciprocal(dn[:], dn[:])
    nc.vector.tensor_mul(tf[:], rt[:], dn[:])
    nc.vector.tensor_mul(
        vv[:].rearrange("b (o l) -> b o l", l=L),
        s[:].rearrange("b (o l) -> b o l", l=L),
        tf[:][:, :, None].broadcast_to([nrows, no, L]))
    return vv


def _build(mode=MODE):
    if mode in _cache:
        return _cache[mode]

    import concourse.bacc as bacc
    import concourse.mybir as mybir
    import concourse.tile as tile

    f32 = mybir.dt.float32
    nc = bacc.Bacc("TRN2", target_bir_lowering=False, debug=False,
                   num_devices=NCORES)
    if mode == "bp":
        return _build_bp(nc, mybir)
    if mode == "bp2":
        return _build_bp2(nc, mybir)
    if mode == "bp3":
        return _build_bp3(nc, mybir)
    xt_d = nc.dram_tensor("xt", [P, B], f32, kind="ExternalInput").ap()
    w_d = nc.dram_tensor("w", [P, LO], f32, kind="ExternalInput").ap()
    out_rows = BB if mode in ("rs", "a2a") else B
    out_d = nc.dram_tensor("out", [out_rows, LO], f32,
                           kind="ExternalOutput").ap()

    with tile.TileContext(nc) as tc:
        with (
            tc.tile_pool(name="io", bufs=3) as io_pool,
            tc.tile_pool(name="ps", bufs=1, space="PSUM") as ps_pool,
            tc.tile_pool(name="dram", bufs=1, space="DRAM") as dram_pool,
            tc.tile_pool(name="post", bufs=1) as post,
        ):
            xt_v = xt_d.rearrange("(c p) b -> c p b", p=128)
            w_v = w_d.rearrange("(c p) f -> c p f", p=128)
            ps0 = ps_pool.tile([128, LO], f32, name="ps0")
            ps1 = ps_pool.tile([128, LO], f32, name="ps1")
            for c in range(KC):
                xt_t = io_pool.tile([128, B], f32, tag="xt", name=f"xt{c}")
                w_t = io_pool.tile([128, LO], f32, tag="w", name=f"w{c}")
                nc.sync.dma_start(xt_t[:], xt_v[c])
                nc.sync.dma_start(w_t[:], w_v[c])
                nc.tensor.matmul(ps0[:], xt_t[:, 0:128], w_t[:],
                                 start=(c == 0), stop=(c == KC - 1))
                nc.tensor.matmul(ps1[:], xt_t[:, 128:256], w_t[:],
                                 start=(c == 0), stop=(c == KC - 1))

            partial = dram_pool.tile([B, LO], f32, name="partial")
            s0 = post.tile([128, LO], f32, name="s0")
            s1 = post.tile([128, LO], f32, name="s1")
            nc.vector.tensor_copy(s0[:], ps0[:])
            nc.vector.tensor_copy(s1[:], ps1[:])
            nc.sync.dma_start(partial[0:128, :], s0[:])
            nc.sync.dma_start(partial[128:256, :], s1[:])

            rg = [list(range(NCORES))]
            if mode == "ar":
                red = dram_pool.tile([B, LO], f32, name="red",
                                     addr_space="Shared")
                nc.gpsimd.collective_compute(
                    "AllReduce", mybir.AluOpType.add, replica_groups=rg,
                    ins=[partial.opt()], outs=[red.opt()])
                for h in range(2):
                    sh = post.tile([128, LO], f32, name=f"sh{h}")
                    nc.sync.dma_start(sh[:], red[128 * h:128 * (h + 1), :])
                    vv = _emit_squash(nc, mybir, post, sh, 128, h)
                    nc.sync.dma_start(out_d[128 * h:128 * (h + 1), :], vv[:])
            elif mode == "ag":
                red = dram_pool.tile([NCORES * B, LO], f32, name="red",
                                     addr_space="Shared")
                nc.gpsimd.collective_compute(
                    "AllGather", mybir.AluOpType.bypass, replica_groups=rg,
                    ins=[partial.opt()], outs=[red.opt()])
                red_v = red.rearrange("(r b) f -> b r f", b=B)
                for h in range(2):
                    r8 = post.tile([128, NCORES, LO], f32, name=f"r8{h}")
                    nc.sync.dma_start(r8[:], red_v[128 * h:128 * (h + 1)])
                    sh = post.tile([128, LO], f32, name=f"sh{h}")
                    nc.vector.reduce_sum(
                        sh[:], r8[:].rearrange("b r f -> b f r"),
                        axis=mybir.AxisListType.X)
                    vv = _emit_squash(nc, mybir, post, sh, 128, h)
                    nc.sync.dma_start(out_d[128 * h:128 * (h + 1), :], vv[:])
            elif mode == "rs":
                red = dram_pool.tile([BB, LO], f32, name="red")
                nc.gpsimd.collective_compute(
                    "ReduceScatter", mybir.AluOpType.add, replica_groups=rg,
                    ins=[partial.opt()], outs=[red.opt()])
                s = post.tile([BB, LO], f32, name="s")
                nc.sync.dma_start(s[:], red[:])
                vv = _emit_squash(nc, mybir, post, s, BB, 0)
                nc.sync.dma_start(out_d[:], vv[:])
            else:  # a2a
                red = dram_pool.tile([B, LO], f32, name="red")
                nc.gpsimd.collective_compute(
                    "AllToAll", mybir.AluOpType.bypass, replica_groups=rg,
                    ins=[partial.opt()], outs=[red.opt()])
                r8 = post.tile([BB, NCORES, LO], f32, name="r8")
                nc.sync.dma_start(r8[:], red.rearrange("(r b) f -> b r f",
                                                       b=BB))
                s = post.tile([BB, LO], f32, name="s")
                nc.vector.reduce_sum(
                    s[:], r8[:].rearrange("b r f -> b f r"),
                    axis=mybir.AxisListType.X)
                vv = _emit_squash(nc, mybir, post, s, BB, 0)
                nc.sync.dma_start(out_d[:], vv[:])

    nc.compile()
    _cache[mode] = nc
    return nc


def _build_bp(nc, mybir):
    """Batch-parallel: W replicated, batch sharded 8 x 32, no collective.

    PE efficiency at M=32 is recovered with 4x column tiling: each PE pass
    runs 4 k-chunks concurrently in the four 32-column groups of the array,
    accumulating into four disjoint 32-partition strips of one PSUM tile.
    The four strips are partial K-sums, added together on DVE at the end.
    DMA is split across both HWDGE queues (sync + scalar)."""
    import concourse.tile as tile

    f32 = mybir.dt.float32
    K = N * P
    xt_d = nc.dram_tensor("xt", [K, BB], f32, kind="ExternalInput").ap()
    w_d = nc.dram_tensor("w", [K, LO], f32, kind="ExternalInput").ap()
    sel_d = nc.dram_tensor("sel", [128, BB], f32, kind="ExternalInput").ap()
    out_d = nc.dram_tensor("out", [BB, LO], f32, kind="ExternalOutput").ap()

    with tile.TileContext(nc) as tc:
        with (
            tc.tile_pool(name="io", bufs=3) as io_pool,
            tc.tile_pool(name="ps", bufs=1, space="PSUM") as ps_pool,
            tc.tile_pool(name="post", bufs=1) as post,
        ):
            xt_v = xt_d.rearrange("(g j p) m -> g p j m", j=GP, p=128)
            w_v = w_d.rearrange("(g j p) f -> g p j f", j=GP, p=128)
            sel_t = post.tile([128, BB], f32, name="sel_t")
            nc.scalar.dma_start(sel_t[:], sel_d[:])
            ps = ps_pool.tile([128, LO], f32, name="ps")
            for g in range(NPASS):
                xt_t = io_pool.tile([128, GP, BB], f32, tag="xt",
                                    name=f"xt{g}")
                w_t = io_pool.tile([128, GP, LO], f32, tag="w", name=f"w{g}")
                dma_eng = nc.sync if g % 2 == 0 else nc.scalar
                xt_eng = nc.scalar if g % 2 == 0 else nc.sync
                xt_eng.dma_start(xt_t[:], xt_v[g])
                dma_eng.dma_start(w_t[:], w_v[g])
                for j in range(GP):
                    nc.tensor.matmul(
                        ps[32 * j:32 * (j + 1), :], xt_t[:, j, :],
                        w_t[:, j, :], start=(g == 0), stop=(g == NPASS - 1),
                        tile_position=(0, 32 * j))

            # sum the four 32-partition strips: s = sel.T @ sp on the PE
            # (DVE cannot add across base partitions; walrus rejects it).
            sp = post.tile([128, LO], f32, name="sp")
            nc.vector.tensor_copy(sp[:], ps[:])
            ps2 = ps_pool.tile([BB, LO], f32, name="ps2")
            nc.tensor.matmul(ps2[:], sel_t[:], sp[:], start=True, stop=True)
            s = post.tile([BB, LO], f32, name="s")
            nc.vector.tensor_copy(s[:], ps2[:])
            vv = _emit_squash(nc, mybir, post, s, BB, 0)
            nc.sync.dma_start(out_d[:], vv[:])

    nc.compile()
    _cache["bp"] = nc
    return nc


def _build_bp2(nc, mybir):
    """Like bp, but inputs are host-packed so each PE pass's W/xt tile is a
    contiguous DRAM block (per-partition runs of 1280B/512B instead of
    640B/128B), and every W pass-load is split across both HWDGE queues."""
    import concourse.tile as tile

    f32 = mybir.dt.float32
    xt_d = nc.dram_tensor("xt", [128, NPASS * GP * BB], f32,
                          kind="ExternalInput").ap()
    w_d = nc.dram_tensor("w", [NPASS * 128, GP * LO], f32,
                         kind="ExternalInput").ap()
    sel_d = nc.dram_tensor("sel", [128, BB], f32, kind="ExternalInput").ap()
    out_d = nc.dram_tensor("out", [BB, LO], f32, kind="ExternalOutput").ap()

    with tile.TileContext(nc) as tc:
        with (
            tc.tile_pool(name="io", bufs=5) as io_pool,
            tc.tile_pool(name="ps", bufs=1, space="PSUM") as ps_pool,
            tc.tile_pool(name="post", bufs=1) as post,
        ):
            # DMA granularity: PR passes per issue (fewer, larger transfers —
            # each dma_start costs ~670ns of issue time on its HWDGE engine,
            # and the kernel-teardown sem storm scales with instruction count).
            # The first group is a single pass so the PE can start sooner.
            PR = 3
            groups = [1] + [PR] * ((NPASS - 1) // PR) + \
                     ([NPASS - 1 - (NPASS - 1) // PR * PR] or [])
            groups = [n for n in groups if n]
            w_vp = w_d.rearrange("(g p) f -> g p f", p=128)
            sel_t = post.tile([128, BB], f32, name="sel_t")
            nc.scalar.dma_start(sel_t[:], sel_d[:])
            # x is tiny (9.2KB/partition): keep it SBUF-resident, loaded by
            # two early DMAs instead of one per group — fewer issues and no
            # xt dependency in the W streaming pipeline.
            XA = 7 * GP * BB
            xt_all = post.tile([128, NPASS * GP * BB], f32, name="xt_all")
            nc.scalar.dma_start(xt_all[:, 0:XA], xt_d[:, 0:XA])
            ps = ps_pool.tile([128, LO], f32, name="ps")
            # PE warm-up: ~4us of dummy matmuls on the tiny sel tile while
            # the first W loads are in flight, so the HAM un-throttles the
            # PE clock (1.2 -> 2.4 GHz) before the real passes start.
            warm = ps_pool.tile([BB, BB], f32, name="warm")
            for _ in range(10):
                nc.tensor.matmul(warm[:], sel_t[:, 0:BB], sel_t[:, 0:BB],
                                 start=True, stop=True)
            g0 = 0
            for gi, npg in enumerate(groups):
                w_t = io_pool.tile([128, npg, GP * LO], f32, tag="w",
                                   name=f"w{gi}")
                ws = w_vp[g0:g0 + npg].rearrange("h p f -> p h f")
                e0, e1 = (nc.sync, nc.scalar) if gi % 2 == 0 else \
                         (nc.scalar, nc.sync)
                if npg == 1:
                    half = GP * LO // 2
                    e0.dma_start(w_t[:, 0, 0:half], ws[:, 0, 0:half])
                    e1.dma_start(w_t[:, 0, half:], ws[:, 0, half:])
                else:
                    # first-needed pass on e0, rest on e1
                    e0.dma_start(w_t[:, 0:1, :], ws[:, 0:1, :])
                    e1.dma_start(w_t[:, 1:npg, :], ws[:, 1:npg, :])
                if gi == 0:
                    nc.sync.dma_start(xt_all[:, XA:], xt_d[:, XA:])
                for h in range(npg):
                    g = g0 + h
                    for j in range(GP):
                        c = g * GP + j
                        nc.tensor.matmul(
                            ps[32 * j:32 * (j + 1), :],
                            xt_all[:, BB * c:BB * (c + 1)],
                            w_t[:, h, LO * j:LO * (j + 1)],
                            start=(g == 0), stop=(g == NPASS - 1),
                            tile_position=(0, 32 * j))
                g0 += npg

            sp = post.tile([128, LO], f32, name="sp")
            nc.vector.tensor_copy(sp[:], ps[:])
            ps2 = ps_pool.tile([BB, LO], f32, name="ps2")
            nc.tensor.matmul(ps2[:], sel_t[:], sp[:], start=True, stop=True)
            s = post.tile([BB, LO], f32, name="s")
            nc.vector.tensor_copy(s[:], ps2[:])
            vv = _emit_squash(nc, mybir, post, s, BB, 0)
            nc.sync.dma_start(out_d[:], vv[:])

    nc.compile()
    _cache["bp2"] = nc
    return nc


def _build_bp3(nc, mybir):
    """4-way batch x 2-way output-capsule sharding, bf16 inputs.

    Each core computes s[b, f] for 64 batch rows and 80 output columns
    (8 of the 16 o-capsules, all 10 l's; the squash l-reduction stays
    core-local).  Per-core traffic drops from 7.1 MB (bp2) to 2.65 MB:
    bf16 halves the bytes and the 4x2 grid replicates x only 2x and W
    only 4x instead of 8x.

    W and x are host-interleaved into ONE packed stream wx: per PE pass
    g the block [w(2 chunks, 160 cols) | xt(2 chunks, 128 cols)], so DMA
    delivery order == PE consumption order, every transfer is one
    contiguous per-partition run, and the two HWDGE queues split each
    group at a pass boundary.  M=64 PE efficiency is recovered with 2x
    column tiling (tile_position=(0,64j)); the two 64-partition strips
    are summed by a small selection-matrix matmul as in bp2.
    """
    import concourse.tile as tile

    f32 = mybir.dt.float32
    bf16 = mybir.dt.bfloat16
    wx_d = nc.dram_tensor("wx", [128, NP3 * WXC], bf16,
                          kind="ExternalInput").ap()
    sel_d = nc.dram_tensor("sel", [128, B4], f32, kind="ExternalInput").ap()
    out_d = nc.dram_tensor("out", [B4, FO], f32, kind="ExternalOutput").ap()

    with tile.TileContext(nc) as tc:
        with (
            tc.tile_pool(name="io", bufs=5) as io_pool,
            tc.tile_pool(name="ps", bufs=1, space="PSUM") as ps_pool,
            tc.tile_pool(name="post", bufs=1) as post,
        ):
            groups = [4, 16, 16]
            assert sum(groups) == NP3
            wx_v = wx_d.rearrange("p (g c) -> p g c", c=WXC)
            # PE warm-up on a memset tile (no DMA dependency) while the
            # first loads are in flight, so the HAM un-throttles the PE
            # clock before the real passes start.
            wsrc = post.tile([128, B4], f32, name="wsrc")
            nc.vector.memset(wsrc[:], 1.0)
            warm = ps_pool.tile([B4, B4], f32, name="warm")
            for _ in range(10):
                nc.tensor.matmul(warm[:], wsrc[:, 0:B4], wsrc[:, 0:B4],
                                 start=True, stop=True)
            sel_t = post.tile([128, B4], f32, name="sel_t")
            ps = ps_pool.tile([128, FO], f32, name="ps")
            g0 = 0
            for gi, npg in enumerate(groups):
                wx_t = io_pool.tile([128, npg, WXC], bf16, tag="wx",
                                    name=f"wx{gi}")
                src = wx_v[:, g0:g0 + npg]
                e0, e1 = (nc.sync, nc.scalar) if gi % 2 == 0 else \
                         (nc.scalar, nc.sync)
                ha = (npg + 1) // 2
                e0.dma_start(wx_t[:, 0:ha, :], src[:, 0:ha])
                e1.dma_start(wx_t[:, ha:npg, :], src[:, ha:npg])
                if gi == 0:
                    # sel is only needed for the final strip-sum; issue it
                    # after the first W/x group so pass-0 data leads.
                    nc.scalar.dma_start(sel_t[:], sel_d[:])
                for h in range(npg):
                    g = g0 + h
                    for j in range(GP3):
                        nc.tensor.matmul(
                            ps[B4 * j:B4 * (j + 1), :],
                            wx_t[:, h, GP3 * FO + B4 * j:
                                 GP3 * FO + B4 * (j + 1)],
                            wx_t[:, h, FO * j:FO * (j + 1)],
                            start=(g == 0), stop=(g == NP3 - 1),
                            tile_position=(0, B4 * j))
                g0 += npg

            # sum the two 64-partition strips: s = sel.T @ sp on the PE
            sp = post.tile([128, FO], f32, name="sp")
            nc.vector.tensor_copy(sp[:], ps[:])
            ps2 = ps_pool.tile([B4, FO], f32, name="ps2")
            nc.tensor.matmul(ps2[:], sel_t[:], sp[:], start=True, stop=True)
            s = post.tile([B4, FO], f32, name="s")
            nc.vector.tensor_copy(s[:], ps2[:])
            vv = _emit_squash(nc, mybir, post, s, B4, 0, no=O2)
            # scalar's issue queue is long done by now; sync still owns
            # the end-barrier bookkeeping, so the out store leaves sooner
            # from scalar.
            nc.scalar.dma_start(out_d[:], vv[:])

    nc.compile()
    _cache["bp3"] = nc
    return nc


def _prep_inputs(x, W, mode=MODE):
    x = np.asarray(x, dtype=np.float32)
    W = np.asarray(W, dtype=np.float32)
    if mode == "bp3":
        import ml_dtypes
        bf16 = ml_dtypes.bfloat16
        # wf rows k=(n,p), cols f=o*10+l
        wf = np.ascontiguousarray(
            W[0].transpose(3, 0, 2, 1).reshape(N * P, LO))
        sel = np.zeros((128, B4), np.float32)
        sel[np.arange(128), np.arange(128) % B4] = 1.0
        # per-pass packed blocks, shared pieces computed once
        wpass = {}
        for ci in range(2):
            wc = wf[:, FO * ci:FO * (ci + 1)].reshape(NP3, GP3 * 128, FO)
            wpass[ci] = wc.reshape(NP3, GP3, 128, FO).transpose(
                0, 2, 1, 3).reshape(NP3, 128, GP3 * FO)
        xpass = {}
        for ri in range(4):
            xt = x[B4 * ri:B4 * (ri + 1)].reshape(B4, N * P).T  # (9216, 64)
            xpass[ri] = xt.reshape(NP3, GP3, 128, B4).transpose(
                0, 2, 1, 3).reshape(NP3, 128, GP3 * B4)
        in_maps = []
        for i in range(NCORES):
            ri, ci = i // 2, i % 2
            wx = np.concatenate([wpass[ci], xpass[ri]], axis=2)
            wx = np.ascontiguousarray(
                wx.transpose(1, 0, 2).reshape(128, NP3 * WXC)).astype(bf16)
            in_maps.append({"wx": wx, "sel": sel})
        return in_maps
    if mode == "bp2":
        # pack so each pass's tile is one contiguous DRAM block:
        # packed[g, p, j*D+d] = flat[128*(GP*g+j)+p, d]
        wf = np.ascontiguousarray(
            W[0].transpose(3, 0, 2, 1).reshape(N * P, LO))
        w2 = np.ascontiguousarray(
            wf.reshape(NPASS, GP, 128, LO).transpose(0, 2, 1, 3)
            .reshape(NPASS * 128, GP * LO))
        sel = np.zeros((128, BB), np.float32)
        sel[np.arange(128), np.arange(128) % BB] = 1.0
        in_maps = []
        for i in range(NCORES):
            xt = x[BB * i:BB * (i + 1)].reshape(BB, N * P).T  # (9216, 32)
            x2 = np.ascontiguousarray(
                xt.reshape(NPASS * GP, 128, BB).transpose(1, 0, 2)
                .reshape(128, NPASS * GP * BB))
            in_maps.append({"xt": x2, "w": w2, "sel": sel})
        return in_maps
    if mode == "bp":
        # xt = per-core batch-slice of x, flattened (b, n*p) and transposed;
        # w = full W with rows k=(n,p), cols f=o*10+l — identical per core.
        wf = np.ascontiguousarray(
            W[0].transpose(3, 0, 2, 1).reshape(N * P, LO))    # (9216, 160)
        sel = np.zeros((128, BB), np.float32)
        sel[np.arange(128), np.arange(128) % BB] = 1.0
        in_maps = []
        for i in range(NCORES):
            xs = x[BB * i:BB * (i + 1)].reshape(BB, N * P)
            in_maps.append({"xt": np.ascontiguousarray(xs.T), "w": wf,
                            "sel": sel})
        return in_maps
    in_maps = []
    for i in range(NCORES):
        xt = np.ascontiguousarray(x[:, i, :].T)               # (1152, 256)
        w = np.ascontiguousarray(
            W[0, :, :, :, i].transpose(0, 2, 1).reshape(P, LO))  # (1152, 160)
        in_maps.append({"xt": xt, "w": w})
    return in_maps


def _postprocess(results, mode=MODE):
    if mode == "bp3":
        full = np.zeros((B, LO), np.float32)
        for i in range(NCORES):
            ri, ci = i // 2, i % 2
            full[B4 * ri:B4 * (ri + 1), FO * ci:FO * (ci + 1)] = \
                results[i]["out"]
        return np.ascontiguousarray(
            full.reshape(B, O, L).transpose(0, 2, 1))
    if mode in ("rs", "a2a", "bp", "bp2"):
        full = np.concatenate([results[i]["out"] for i in range(NCORES)],
                              axis=0)
    else:
        full = results[0]["out"]
    return np.ascontiguousarray(
        full.reshape(B, O, L).transpose(0, 2, 1))             # (256, 10, 16)


def kernel(x, W):
    from concourse.bass_utils import run_bass_kernel_spmd

    nc = _build(MODE)
    res = run_bass_kernel_spmd(nc, _prep_inputs(x, W, MODE),
                               core_ids=list(range(NCORES)))
    return _postprocess(res.results)



# revision 12
# speedup vs baseline: 1.5529x; 1.0463x over previous
"""Trainium2 Bass kernel for nn_DigitCap (capsule DigitCaps layer).

Math: the reference's routing loop is degenerate — softmax over a size-1
axis is exactly 1.0, so c_ij == 1 on every iteration and the output only
depends on s[b,l,o] = sum_{p,n} W[0,p,l,o,n] * x[b,n,p], followed by the
squash nonlinearity (norm taken over the L axis, faithful to the source):

    m2[b,o]    = sum_l s[b,l,o]^2
    out[b,l,o] = s[b,l,o] * sqrt(m2[b,o]) / (1 + m2[b,o])

This collapses to one (256 x 9216) @ (9216 x 160) matmul plus a tiny
elementwise epilogue.

Sharding over 8 NeuronCores — shipped mode "bp2", batch-parallel with NO
collective: on this stack every 8-rank collective costs 50-65us of ncfw
control-plane latency regardless of payload (measured AR/AG/RS/A2A), which
dwarfs the extra DMA of replicating W.  So each core takes 32 batch rows,
reads all of W (5.9 MB) plus its 1.2 MB x-slice, and no cross-core
communication happens at all.

PE efficiency at M=32 is recovered with 4x column tiling: each PE pass
runs 4 K-chunks concurrently in the four 32-column groups of the array
(tile_position=(0,32j)), accumulating into four disjoint 32-partition
strips of one PSUM tile; the strips are then combined with a tiny
selection-matrix matmul (DVE cannot add across base partitions).  Inputs
are host-packed so each pass's W/xt tile is one contiguous DRAM block,
and every W pass-load is split across both HWDGE queues (sync+scalar),
which lifts aggregate DMA from ~190 to ~245 GB/s — the matmul phase is
DMA-bandwidth-bound (7.1 MB/core through the LNC1-shared HBM port).

Alternate modes kept for reference (all measured slower): "bp" (unpacked
batch-parallel, 52us), "a2a" (K-sharded + AllToAll, 87us), "rs"/"ar"/"ag"
(K-sharded + ReduceScatter/AllReduce/AllGather, 96-105us); shipped bp2
measures ~38us end-to-end on hardware (paired DMA issues, SBUF-resident
x loaded by two early DMAs, PE warm-up
matmuls during the load lead-in, balanced queue alternation).

The free dim everywhere is ordered f = o*10 + l so the squash l-reduction
is an innermost-axis DVE reduce; the host converts the gathered (256,160)
result back to (256, 10, 16).
"""

import numpy as np

B, N, P, L, O = 256, 8, 1152, 10, 16
NCORES = 8
KC = P // 128          # 9 k-chunks of 128 per core
BB = B // NCORES       # 32 batch rows per core in the scatter modes
LO = L * O             # 160

MODE = "bp3"

GP = 4                 # col-tiled k-chunks per PE pass in "bp" mode
NPASS = N * P // 128 // GP   # 18 passes over the full K for one core

# bp3: 4-way batch x 2-way output-capsule sharding, bf16 inputs.
B4 = B // 4            # 64 batch rows per core
O2 = O // 2            # 8 output capsules per core
FO = O2 * L            # 80 output columns per core (f = o_local*10 + l)
KC3 = N * P // 128     # 72 k-chunks of 128
GP3 = 2                # col-tiled k-chunks per PE pass (two 64-col groups)
NP3 = KC3 // GP3       # 36 passes
WXC = GP3 * FO + GP3 * B4   # 288 packed cols per pass: [w | xt]

_cache = {}


def _emit_squash(nc, mybir, post, s, nrows, idx, no=O):
    """Emit squash for an SBUF tile s of shape [nrows, no*L]; returns v tile."""
    f32 = mybir.dt.float32
    nf = no * L
    sq = post.tile([nrows, nf], f32, name=f"sq{idx}")
    m2 = post.tile([nrows, no], f32, name=f"m2{idx}")
    rt = post.tile([nrows, no], f32, name=f"rt{idx}")
    dn = post.tile([nrows, no], f32, name=f"dn{idx}")
    tf = post.tile([nrows, no], f32, name=f"tf{idx}")
    vv = post.tile([nrows, nf], f32, name=f"vv{idx}")
    nc.vector.tensor_mul(sq[:], s[:], s[:])
    nc.vector.reduce_sum(
        m2[:], sq[:].rearrange("b (o l) -> b o l", l=L),
        axis=mybir.AxisListType.X)
    nc.scalar.activation(rt[:], m2[:], mybir.ActivationFunctionType.Sqrt)
    nc.vector.tensor_scalar_add(dn[:], m2[:], 1.0)
    nc.vector.reciprocal(dn[:], dn[:])
    nc.vector.tensor_mul(tf[:], rt[:], dn[:])
    nc.vector.tensor_mul(
        vv[:].rearrange("b (o l) -> b o l", l=L),
        s[:].rearrange("b (o l) -> b o l", l=L),
        tf[:][:, :, None].broadcast_to([nrows, no, L]))
    return vv


def _build(mode=MODE):
    if mode in _cache:
        return _cache[mode]

    import concourse.bacc as bacc
    import concourse.mybir as mybir
    import concourse.tile as tile

    f32 = mybir.dt.float32
    nc = bacc.Bacc("TRN2", target_bir_lowering=False, debug=False,
                   num_devices=NCORES)
    if mode == "bp":
        return _build_bp(nc, mybir)
    if mode == "bp2":
        return _build_bp2(nc, mybir)
    if mode == "bp3":
        return _build_bp3(nc, mybir)
    xt_d = nc.dram_tensor("xt", [P, B], f32, kind="ExternalInput").ap()
    w_d = nc.dram_tensor("w", [P, LO], f32, kind="ExternalInput").ap()
    out_rows = BB if mode in ("rs", "a2a") else B
    out_d = nc.dram_tensor("out", [out_rows, LO], f32,
                           kind="ExternalOutput").ap()

    with tile.TileContext(nc) as tc:
        with (
            tc.tile_pool(name="io", bufs=3) as io_pool,
            tc.tile_pool(name="ps", bufs=1, space="PSUM") as ps_pool,
            tc.tile_pool(name="dram", bufs=1, space="DRAM") as dram_pool,
            tc.tile_pool(name="post", bufs=1) as post,
        ):
            xt_v = xt_d.rearrange("(c p) b -> c p b", p=128)
            w_v = w_d.rearrange("(c p) f -> c p f", p=128)
            ps0 = ps_pool.tile([128, LO], f32, name="ps0")
            ps1 = ps_pool.tile([128, LO], f32, name="ps1")
            for c in range(KC):
                xt_t = io_pool.tile([128, B], f32, tag="xt", name=f"xt{c}")
                w_t = io_pool.tile([128, LO], f32, tag="w", name=f"w{c}")
                nc.sync.dma_start(xt_t[:], xt_v[c])
                nc.sync.dma_start(w_t[:], w_v[c])
                nc.tensor.matmul(ps0[:], xt_t[:, 0:128], w_t[:],
                                 start=(c == 0), stop=(c == KC - 1))
                nc.tensor.matmul(ps1[:], xt_t[:, 128:256], w_t[:],
                                 start=(c == 0), stop=(c == KC - 1))

            partial = dram_pool.tile([B, LO], f32, name="partial")
            s0 = post.tile([128, LO], f32, name="s0")
            s1 = post.tile([128, LO], f32, name="s1")
            nc.vector.tensor_copy(s0[:], ps0[:])
            nc.vector.tensor_copy(s1[:], ps1[:])
            nc.sync.dma_start(partial[0:128, :], s0[:])
            nc.sync.dma_start(partial[128:256, :], s1[:])

            rg = [list(range(NCORES))]
            if mode == "ar":
                red = dram_pool.tile([B, LO], f32, name="red",
                                     addr_space="Shared")
                nc.gpsimd.collective_compute(
                    "AllReduce", mybir.AluOpType.add, replica_groups=rg,
                    ins=[partial.opt()], outs=[red.opt()])
                for h in range(2):
                    sh = post.tile([128, LO], f32, name=f"sh{h}")
                    nc.sync.dma_start(sh[:], red[128 * h:128 * (h + 1), :])
                    vv = _emit_squash(nc, mybir, post, sh, 128, h)
                    nc.sync.dma_start(out_d[128 * h:128 * (h + 1), :], vv[:])
            elif mode == "ag":
                red = dram_pool.tile([NCORES * B, LO], f32, name="red",
                                     addr_space="Shared")
                nc.gpsimd.collective_compute(
                    "AllGather", mybir.AluOpType.bypass, replica_groups=rg,
                    ins=[partial.opt()], outs=[red.opt()])
                red_v = red.rearrange("(r b) f -> b r f", b=B)
                for h in range(2):
                    r8 = post.tile([128, NCORES, LO], f32, name=f"r8{h}")
                    nc.sync.dma_start(r8[:], red_v[128 * h:128 * (h + 1)])
                    sh = post.tile([128, LO], f32, name=f"sh{h}")
                    nc.vector.reduce_sum(
                        sh[:], r8[:].rearrange("b r f -> b f r"),
                        axis=mybir.AxisListType.X)
                    vv = _emit_squash(nc, mybir, post, sh, 128, h)
                    nc.sync.dma_start(out_d[128 * h:128 * (h + 1), :], vv[:])
            elif mode == "rs":
                red = dram_pool.tile([BB, LO], f32, name="red")
                nc.gpsimd.collective_compute(
                    "ReduceScatter", mybir.AluOpType.add, replica_groups=rg,
                    ins=[partial.opt()], outs=[red.opt()])
                s = post.tile([BB, LO], f32, name="s")
                nc.sync.dma_start(s[:], red[:])
                vv = _emit_squash(nc, mybir, post, s, BB, 0)
                nc.sync.dma_start(out_d[:], vv[:])
            else:  # a2a
                red = dram_pool.tile([B, LO], f32, name="red")
                nc.gpsimd.collective_compute(
                    "AllToAll", mybir.AluOpType.bypass, replica_groups=rg,
                    ins=[partial.opt()], outs=[red.opt()])
                r8 = post.tile([BB, NCORES, LO], f32, name="r8")
                nc.sync.dma_start(r8[:], red.rearrange("(r b) f -> b r f",
                                                       b=BB))
                s = post.tile([BB, LO], f32, name="s")
                nc.vector.reduce_sum(
                    s[:], r8[:].rearrange("b r f -> b f r"),
                    axis=mybir.AxisListType.X)
                vv = _emit_squash(nc, mybir, post, s, BB, 0)
                nc.sync.dma_start(out_d[:], vv[:])

    nc.compile()
    _cache[mode] = nc
    return nc


def _build_bp(nc, mybir):
    """Batch-parallel: W replicated, batch sharded 8 x 32, no collective.

    PE efficiency at M=32 is recovered with 4x column tiling: each PE pass
    runs 4 k-chunks concurrently in the four 32-column groups of the array,
    accumulating into four disjoint 32-partition strips of one PSUM tile.
    The four strips are partial K-sums, added together on DVE at the end.
    DMA is split across both HWDGE queues (sync + scalar)."""
    import concourse.tile as tile

    f32 = mybir.dt.float32
    K = N * P
    xt_d = nc.dram_tensor("xt", [K, BB], f32, kind="ExternalInput").ap()
    w_d = nc.dram_tensor("w", [K, LO], f32, kind="ExternalInput").ap()
    sel_d = nc.dram_tensor("sel", [128, BB], f32, kind="ExternalInput").ap()
    out_d = nc.dram_tensor("out", [BB, LO], f32, kind="ExternalOutput").ap()

    with tile.TileContext(nc) as tc:
        with (
            tc.tile_pool(name="io", bufs=3) as io_pool,
            tc.tile_pool(name="ps", bufs=1, space="PSUM") as ps_pool,
            tc.tile_pool(name="post", bufs=1) as post,
        ):
            xt_v = xt_d.rearrange("(g j p) m -> g p j m", j=GP, p=128)
            w_v = w_d.rearrange("(g j p) f -> g p j f", j=GP, p=128)
            sel_t = post.tile([128, BB], f32, name="sel_t")
            nc.scalar.dma_start(sel_t[:], sel_d[:])
            ps = ps_pool.tile([128, LO], f32, name="ps")
            for g in range(NPASS):
                xt_t = io_pool.tile([128, GP, BB], f32, tag="xt",
                                    name=f"xt{g}")
                w_t = io_pool.tile([128, GP, LO], f32, tag="w", name=f"w{g}")
                dma_eng = nc.sync if g % 2 == 0 else nc.scalar
                xt_eng = nc.scalar if g % 2 == 0 else nc.sync
                xt_eng.dma_start(xt_t[:], xt_v[g])
                dma_eng.dma_start(w_t[:], w_v[g])
                for j in range(GP):
                    nc.tensor.matmul(
                        ps[32 * j:32 * (j + 1), :], xt_t[:, j, :],
                        w_t[:, j, :], start=(g == 0), stop=(g == NPASS - 1),
                        tile_position=(0, 32 * j))

            # sum the four 32-partition strips: s = sel.T @ sp on the PE
            # (DVE cannot add across base partitions; walrus rejects it).
            sp = post.tile([128, LO], f32, name="sp")
            nc.vector.tensor_copy(sp[:], ps[:])
            ps2 = ps_pool.tile([BB, LO], f32, name="ps2")
            nc.tensor.matmul(ps2[:], sel_t[:], sp[:], start=True, stop=True)
            s = post.tile([BB, LO], f32, name="s")
            nc.vector.tensor_copy(s[:], ps2[:])
            vv = _emit_squash(nc, mybir, post, s, BB, 0)
            nc.sync.dma_start(out_d[:], vv[:])

    nc.compile()
    _cache["bp"] = nc
    return nc


def _build_bp2(nc, mybir):
    """Like bp, but inputs are host-packed so each PE pass's W/xt tile is a
    contiguous DRAM block (per-partition runs of 1280B/512B instead of
    640B/128B), and every W pass-load is split across both HWDGE queues."""
    import concourse.tile as tile

    f32 = mybir.dt.float32
    xt_d = nc.dram_tensor("xt", [128, NPASS * GP * BB], f32,
                          kind="ExternalInput").ap()
    w_d = nc.dram_tensor("w", [NPASS * 128, GP * LO], f32,
                         kind="ExternalInput").ap()
    sel_d = nc.dram_tensor("sel", [128, BB], f32, kind="ExternalInput").ap()
    out_d = nc.dram_tensor("out", [BB, LO], f32, kind="ExternalOutput").ap()

    with tile.TileContext(nc) as tc:
        with (
            tc.tile_pool(name="io", bufs=5) as io_pool,
            tc.tile_pool(name="ps", bufs=1, space="PSUM") as ps_pool,
            tc.tile_pool(name="post", bufs=1) as post,
        ):
            # DMA granularity: PR passes per issue (fewer, larger transfers —
            # each dma_start costs ~670ns of issue time on its HWDGE engine,
            # and the kernel-teardown sem storm scales with instruction count).
            # The first group is a single pass so the PE can start sooner.
            PR = 3
            groups = [1] + [PR] * ((NPASS - 1) // PR) + \
                     ([NPASS - 1 - (NPASS - 1) // PR * PR] or [])
            groups = [n for n in groups if n]
            w_vp = w_d.rearrange("(g p) f -> g p f", p=128)
            sel_t = post.tile([128, BB], f32, name="sel_t")
            nc.scalar.dma_start(sel_t[:], sel_d[:])
            # x is tiny (9.2KB/partition): keep it SBUF-resident, loaded by
            # two early DMAs instead of one per group — fewer issues and no
            # xt dependency in the W streaming pipeline.
            XA = 7 * GP * BB
            xt_all = post.tile([128, NPASS * GP * BB], f32, name="xt_all")
            nc.scalar.dma_start(xt_all[:, 0:XA], xt_d[:, 0:XA])
            ps = ps_pool.tile([128, LO], f32, name="ps")
            # PE warm-up: ~4us of dummy matmuls on the tiny sel tile while
            # the first W loads are in flight, so the HAM un-throttles the
            # PE clock (1.2 -> 2.4 GHz) before the real passes start.
            warm = ps_pool.tile([BB, BB], f32, name="warm")
            for _ in range(10):
                nc.tensor.matmul(warm[:], sel_t[:, 0:BB], sel_t[:, 0:BB],
                                 start=True, stop=True)
            g0 = 0
            for gi, npg in enumerate(groups):
                w_t = io_pool.tile([128, npg, GP * LO], f32, tag="w",
                                   name=f"w{gi}")
                ws = w_vp[g0:g0 + npg].rearrange("h p f -> p h f")
                e0, e1 = (nc.sync, nc.scalar) if gi % 2 == 0 else \
                         (nc.scalar, nc.sync)
                if npg == 1:
                    half = GP * LO // 2
                    e0.dma_start(w_t[:, 0, 0:half], ws[:, 0, 0:half])
                    e1.dma_start(w_t[:, 0, half:], ws[:, 0, half:])
                else:
                    # first-needed pass on e0, rest on e1
                    e0.dma_start(w_t[:, 0:1, :], ws[:, 0:1, :])
                    e1.dma_start(w_t[:, 1:npg, :], ws[:, 1:npg, :])
                if gi == 0:
                    nc.sync.dma_start(xt_all[:, XA:], xt_d[:, XA:])
                for h in range(npg):
                    g = g0 + h
                    for j in range(GP):
                        c = g * GP + j
                        nc.tensor.matmul(
                            ps[32 * j:32 * (j + 1), :],
                            xt_all[:, BB * c:BB * (c + 1)],
                            w_t[:, h, LO * j:LO * (j + 1)],
                            start=(g == 0), stop=(g == NPASS - 1),
                            tile_position=(0, 32 * j))
                g0 += npg

            sp = post.tile([128, LO], f32, name="sp")
            nc.vector.tensor_copy(sp[:], ps[:])
            ps2 = ps_pool.tile([BB, LO], f32, name="ps2")
            nc.tensor.matmul(ps2[:], sel_t[:], sp[:], start=True, stop=True)
            s = post.tile([BB, LO], f32, name="s")
            nc.vector.tensor_copy(s[:], ps2[:])
            vv = _emit_squash(nc, mybir, post, s, BB, 0)
            nc.sync.dma_start(out_d[:], vv[:])

    nc.compile()
    _cache["bp2"] = nc
    return nc


def _build_bp3(nc, mybir):
    """4-way batch x 2-way output-capsule sharding, bf16 inputs.

    Each core computes s[b, f] for 64 batch rows and 80 output columns
    (8 of the 16 o-capsules, all 10 l's; the squash l-reduction stays
    core-local).  Per-core traffic drops from 7.1 MB (bp2) to 2.65 MB:
    bf16 halves the bytes and the 4x2 grid replicates x only 2x and W
    only 4x instead of 8x.

    W and x are host-interleaved into ONE packed stream wx: per PE pass
    g the block [w(2 chunks, 160 cols) | xt(2 chunks, 128 cols)], so DMA
    delivery order == PE consumption order, every transfer is one
    contiguous per-partition run, and the two HWDGE queues split each
    group at a pass boundary.  M=64 PE efficiency is recovered with 2x
    column tiling (tile_position=(0,64j)); the two 64-partition strips
    are summed by a small selection-matrix matmul as in bp2.
    """
    import concourse.tile as tile

    f32 = mybir.dt.float32
    bf16 = mybir.dt.bfloat16
    wx_d = nc.dram_tensor("wx", [128, NP3 * WXC], bf16,
                          kind="ExternalInput").ap()
    sel_d = nc.dram_tensor("sel", [128, B4], bf16, kind="ExternalInput").ap()
    out_d = nc.dram_tensor("out", [B4, FO], f32, kind="ExternalOutput").ap()

    with tile.TileContext(nc) as tc:
        with (
            tc.tile_pool(name="io", bufs=5) as io_pool,
            tc.tile_pool(name="ps", bufs=1, space="PSUM") as ps_pool,
            tc.tile_pool(name="post", bufs=1) as post,
        ):
            # Sequential small groups: one dma_start per group (128
            # descriptors with npg*576B contiguous runs), engines
            # alternating, so PE unblocks every ~6 passes instead of
            # every 16 and the exposed tail after the last transfer is
            # ~0.8us.  No warm-up: the HAM never un-throttles the PE
            # clock mid-kernel (bp2's ramp fired at t=33us, after its
            # matmuls), so warm-up matmuls only delayed the first pass.
            groups = [2, 4] + [6] * 5
            assert sum(groups) == NP3
            wx_v = wx_d.rearrange("p (g c) -> p g c", c=WXC)
            sel_t = post.tile([128, B4], bf16, name="sel_t")
            ps = ps_pool.tile([128, FO], f32, name="ps")
            g0 = 0
            for gi, npg in enumerate(groups):
                wx_t = io_pool.tile([128, npg, WXC], bf16, tag="wx",
                                    name=f"wx{gi}")
                e = nc.sync if gi % 2 == 0 else nc.scalar
                e.dma_start(wx_t[:], wx_v[:, g0:g0 + npg])
                if gi == 0:
                    # sel is only needed for the final strip-sum; issue it
                    # after the first W/x group so pass-0 data leads.
                    nc.scalar.dma_start(sel_t[:], sel_d[:])
                for h in range(npg):
                    g = g0 + h
                    for j in range(GP3):
                        nc.tensor.matmul(
                            ps[B4 * j:B4 * (j + 1), :],
                            wx_t[:, h, GP3 * FO + B4 * j:
                                 GP3 * FO + B4 * (j + 1)],
                            wx_t[:, h, FO * j:FO * (j + 1)],
                            start=(g == 0), stop=(g == NP3 - 1),
                            tile_position=(0, B4 * j))
                g0 += npg

            # sum the two 64-partition strips: s = sel.T @ sp on the PE
            # (bf16 so the strip-sum stream runs at full rate)
            sp = post.tile([128, FO], bf16, name="sp")
            nc.vector.tensor_copy(sp[:], ps[:])
            ps2 = ps_pool.tile([B4, FO], f32, name="ps2")
            nc.tensor.matmul(ps2[:], sel_t[:], sp[:], start=True, stop=True)
            s = post.tile([B4, FO], f32, name="s")
            nc.vector.tensor_copy(s[:], ps2[:])
            vv = _emit_squash(nc, mybir, post, s, B4, 0, no=O2)
            # scalar's issue queue is long done by now; sync still owns
            # the end-barrier bookkeeping, so the out store leaves sooner
            # from scalar.
            nc.scalar.dma_start(out_d[:], vv[:])

    nc.compile()
    _cache["bp3"] = nc
    return nc


def _prep_inputs(x, W, mode=MODE):
    x = np.asarray(x, dtype=np.float32)
    W = np.asarray(W, dtype=np.float32)
    if mode == "bp3":
        import ml_dtypes
        bf16 = ml_dtypes.bfloat16
        # wf rows k=(n,p), cols f=o*10+l
        wf = np.ascontiguousarray(
            W[0].transpose(3, 0, 2, 1).reshape(N * P, LO))
        sel = np.zeros((128, B4), np.float32)
        sel[np.arange(128), np.arange(128) % B4] = 1.0
        sel = sel.astype(bf16)
        # per-pass packed blocks, shared pieces computed once
        wpass = {}
        for ci in range(2):
            wc = wf[:, FO * ci:FO * (ci + 1)].reshape(NP3, GP3 * 128, FO)
            wpass[ci] = wc.reshape(NP3, GP3, 128, FO).transpose(
                0, 2, 1, 3).reshape(NP3, 128, GP3 * FO)
        xpass = {}
        for ri in range(4):
            xt = x[B4 * ri:B4 * (ri + 1)].reshape(B4, N * P).T  # (9216, 64)
            xpass[ri] = xt.reshape(NP3, GP3, 128, B4).transpose(
                0, 2, 1, 3).reshape(NP3, 128, GP3 * B4)
        in_maps = []
        for i in range(NCORES):
            ri, ci = i // 2, i % 2
            wx = np.concatenate([wpass[ci], xpass[ri]], axis=2)
            wx = np.ascontiguousarray(
                wx.transpose(1, 0, 2).reshape(128, NP3 * WXC)).astype(bf16)
            in_maps.append({"wx": wx, "sel": sel})
        return in_maps
    if mode == "bp2":
        # pack so each pass's tile is one contiguous DRAM block:
        # packed[g, p, j*D+d] = flat[128*(GP*g+j)+p, d]
        wf = np.ascontiguousarray(
            W[0].transpose(3, 0, 2, 1).reshape(N * P, LO))
        w2 = np.ascontiguousarray(
            wf.reshape(NPASS, GP, 128, LO).transpose(0, 2, 1, 3)
            .reshape(NPASS * 128, GP * LO))
        sel = np.zeros((128, BB), np.float32)
        sel[np.arange(128), np.arange(128) % BB] = 1.0
        in_maps = []
        for i in range(NCORES):
            xt = x[BB * i:BB * (i + 1)].reshape(BB, N * P).T  # (9216, 32)
            x2 = np.ascontiguousarray(
                xt.reshape(NPASS * GP, 128, BB).transpose(1, 0, 2)
                .reshape(128, NPASS * GP * BB))
            in_maps.append({"xt": x2, "w": w2, "sel": sel})
        return in_maps
    if mode == "bp":
        # xt = per-core batch-slice of x, flattened (b, n*p) and transposed;
        # w = full W with rows k=(n,p), cols f=o*10+l — identical per core.
        wf = np.ascontiguousarray(
            W[0].transpose(3, 0, 2, 1).reshape(N * P, LO))    # (9216, 160)
        sel = np.zeros((128, BB), np.float32)
        sel[np.arange(128), np.arange(128) % BB] = 1.0
        in_maps = []
        for i in range(NCORES):
            xs = x[BB * i:BB * (i + 1)].reshape(BB, N * P)
            in_maps.append({"xt": np.ascontiguousarray(xs.T), "w": wf,
                            "sel": sel})
        return in_maps
    in_maps = []
    for i in range(NCORES):
        xt = np.ascontiguousarray(x[:, i, :].T)               # (1152, 256)
        w = np.ascontiguousarray(
            W[0, :, :, :, i].transpose(0, 2, 1).reshape(P, LO))  # (1152, 160)
        in_maps.append({"xt": xt, "w": w})
    return in_maps


def _postprocess(results, mode=MODE):
    if mode == "bp3":
        full = np.zeros((B, LO), np.float32)
        for i in range(NCORES):
            ri, ci = i // 2, i % 2
            full[B4 * ri:B4 * (ri + 1), FO * ci:FO * (ci + 1)] = \
                results[i]["out"]
        return np.ascontiguousarray(
            full.reshape(B, O, L).transpose(0, 2, 1))
    if mode in ("rs", "a2a", "bp", "bp2"):
        full = np.concatenate([results[i]["out"] for i in range(NCORES)],
                              axis=0)
    else:
        full = results[0]["out"]
    return np.ascontiguousarray(
        full.reshape(B, O, L).transpose(0, 2, 1))             # (256, 10, 16)


def kernel(x, W):
    from concourse.bass_utils import run_bass_kernel_spmd

    nc = _build(MODE)
    res = run_bass_kernel_spmd(nc, _prep_inputs(x, W, MODE),
                               core_ids=list(range(NCORES)))
    return _postprocess(res.results)

